# revision 16
# baseline (speedup 1.0000x reference)
"""Causal self-attention (B=2, S=2048, HID=1024, 16 heads x 64) on 8 trn2
NeuronCores.

Sharding: data-parallel over batch (cores 0-3 -> batch 0, cores 4-7 ->
batch 1), tensor-parallel over heads (4 heads per core via Wqk/Wv column
slices). Each core computes its 4 heads end-to-end; the [S, S] score
matrix stays core-local.

Per-core layout choices:
  - q, k are produced TRANSPOSED ([head_cols, S]) so score matmuls need
    no on-device transposes; scores are computed transposed ([sk, sq])
    so the P @ v matmul consumes exp(scores) directly from SBUF.
  - v carries an appended ones-column per head; the attention output
    matmul then yields softmax row-sums in an extra partition row for
    free (no max-subtraction is needed: scores are O(5) so exp is safe
    in fp32, and masked entries are zeroed multiplicatively post-exp).
  - All matmuls run in float32r (fp32 data, PE fast path).
  - Heads are processed in pairs: the two K=64 score matmuls sit in PE
    row-groups 0-63 / 64-127 and run concurrently in the array.
  - The second half (S columns 1024:2048) of the q/k/v projections is
    emitted interleaved with the attention over stripes a=0,1 (which
    only need the first half), so the tensor engine never idles while
    the scalar engine works through the exp() stream -- idle windows
    re-throttle the PE clock to 1.2 GHz (HAM).
"""
import sys

for _p in ("/opt/trn_rl_repo",):
    if _p not in sys.path:
        sys.path.insert(0, _p)

import numpy as np

B, S, HID = 2, 2048, 1024
NH, HD = 16, 64
NHL = 4            # heads per core
WC = NHL * HD      # 256 local q/k weight cols
VC = NHL * (HD + 1)  # 260 local v cols incl. ones col
NT = S // 128      # 16 key chunks
NA = S // 512      # 4 query stripes
NK = HID // 128    # 8 contraction chunks

_NC = None


def _build():
    from concourse import bacc, mybir
    from concourse.tile import TileContext
    from concourse.masks import make_identity

    FP = mybir.dt.float32
    FPR = mybir.dt.float32r
    Exp = mybir.ActivationFunctionType.Exp

    nc = bacc.Bacc("TRN2", target_bir_lowering=False, debug=False, num_devices=8)

    xT = nc.dram_tensor("xT", [HID, S], FPR, kind="ExternalInput")
    wq = nc.dram_tensor("wq", [HID, WC], FPR, kind="ExternalInput")
    wk = nc.dram_tensor("wk", [HID, WC], FPR, kind="ExternalInput")
    wv = nc.dram_tensor("wv", [HID + 1, VC], FPR, kind="ExternalInput")
    bq = nc.dram_tensor("bq", [WC, 1], FP, kind="ExternalInput")
    bk = nc.dram_tensor("bk", [WC, 1], FP, kind="ExternalInput")
    ones = nc.dram_tensor("ones", [1, 128], FPR, kind="ExternalInput")
    out = nc.dram_tensor("out", [S, WC], FP, kind="ExternalOutput")

    with TileContext(nc) as tc:
        with (
            tc.tile_pool(name="inp", bufs=1) as inp,
            tc.tile_pool(name="ptp", bufs=4) as ptp,
            tc.tile_pool(name="osb", bufs=2) as osb,
            tc.tile_pool(name="rcp", bufs=4) as rcp,
            tc.tile_pool(name="onat", bufs=8) as onp,
            tc.tile_pool(name="G", bufs=3, space="PSUM") as gp,
            tc.tile_pool(name="oT", bufs=2, space="PSUM") as otp,
        ):
            # ---- persistent inputs in SBUF ----
            wq_k = [inp.tile([128, WC], FPR, name=f"wq{k}") for k in range(NK)]
            for k in range(NK):
                nc.sync.dma_start(wq_k[k][:, :], wq[k * 128:(k + 1) * 128, :])
            # x.T, column-grouped (nth = 1024-wide halves of S) so the first
            # projection matmuls start after 4MB of DMA, not 8MB
            xk = [[None] * 2 for _ in range(NK)]
            for nth in range(2):
                for k in range(NK):
                    t = inp.tile([128, 1024], FPR, name=f"x{k}_{nth}")
                    nc.sync.dma_start(
                        t[:, :], xT[k * 128:(k + 1) * 128, nth * 1024:(nth + 1) * 1024]
                    )
                    xk[k][nth] = t
                if nth == 0:
                    wk_k = [inp.tile([128, WC], FPR, name=f"wk{k}") for k in range(NK)]
                    for k in range(NK):
                        nc.sync.dma_start(wk_k[k][:, :], wk[k * 128:(k + 1) * 128, :])
                    # v weights must land before the first-half v projections
                    # (they gate attention stripe a=0)
                    wv_k = [inp.tile([128, VC], FPR, name=f"wv{k}")
                            for k in range(NK)]
                    for k in range(NK):
                        nc.sync.dma_start(wv_k[k][:, :], wv[k * 128:(k + 1) * 128, :])
                    wv_last = inp.tile([1, VC], FPR, name="wvl")
                    nc.sync.dma_start(wv_last[:, :], wv[HID:HID + 1, :])
            bq_sb = [inp.tile([128, 1], FP, name=f"bq{t}") for t in range(2)]
            bk_sb = [inp.tile([128, 1], FP, name=f"bk{t}") for t in range(2)]
            for t in range(2):
                nc.sync.dma_start(bq_sb[t][:, :], bq[t * 128:(t + 1) * 128, :])
                nc.sync.dma_start(bk_sb[t][:, :], bk[t * 128:(t + 1) * 128, :])
            ones1 = inp.tile([1, 128], FPR, name="ones1")
            nc.sync.dma_start(ones1[:, :], ones[:, :])
            ident = inp.tile([128, 128], FP, name="ident")
            make_identity(nc, ident[:, :])

            # split by S-half (nth) so interleaved second-half projection
            # writes can't false-depend against first-half attention reads
            qT_sb = [[inp.tile([128, 1024], FPR, name=f"qT{t}_{n}")
                      for n in range(2)] for t in range(2)]
            kT_sb = [[inp.tile([128, 1024], FPR, name=f"kT{t}_{n}")
                      for n in range(2)] for t in range(2)]
            v_sb = [inp.tile([128, VC], FPR, name=f"v{c}") for c in range(NT)]

            # ---- projection emitters ----
            def proj_qk_unit(wt, bt, dst, t, nth):
                g = gp.tile([128, 1024], mybir.dt.float32, tag="G", name="g")
                for k in range(NK):
                    for sub in range(2):
                        nc.tensor.matmul(
                            g[:, sub * 512:(sub + 1) * 512],
                            lhsT=wt[k][:, t * 128:(t + 1) * 128],
                            rhs=xk[k][nth][:, sub * 512:(sub + 1) * 512],
                            start=(k == 0), stop=(k == NK - 1),
                        )
                nc.vector.tensor_scalar_add(
                    dst[t][nth][:, :], g[:, :], bt[t][:, :]
                )

            def proj_v_unit(c):
                nth, cc = divmod(c, 8)
                g = gp.tile([128, 1024], mybir.dt.float32, tag="G", name="g")
                for k in range(NK):
                    nc.tensor.matmul(
                        g[:, :VC],
                        lhsT=xk[k][nth][:, cc * 128:(cc + 1) * 128],
                        rhs=wv_k[k][:, :],
                        start=(k == 0), stop=False,
                    )
                nc.tensor.matmul(  # bias row + ones column (K=1)
                    g[:, :VC], lhsT=ones1[:, :], rhs=wv_last[:, :],
                    start=False, stop=True,
                )
                nc.vector.tensor_copy(v_sb[c][:, :], g[:, :VC])

            # ---- attention emitters ----
            # unit = ONE key chunk b for a head PAIR: g = [h0-slice | h1-slice],
            # one exp covers both heads; fine granularity keeps 3 chunks in
            # flight within the 6 PSUM banks of the G pool
            def att_unit(a, ht, b, nchunks, oTs):
                g = gp.tile([128, 1024], mybir.dt.float32, tag="G", name="g")
                kn, ko = divmod(b * 128, 1024)
                qn, qo = divmod(a * 512, 1024)
                for hh in range(2):
                    hb = hh * 64
                    nc.tensor.matmul(
                        g[:, hh * 512:(hh + 1) * 512],
                        lhsT=kT_sb[ht][kn][hb:hb + 64, ko:ko + 128],
                        rhs=qT_sb[ht][qn][hb:hb + 64, qo:qo + 512],
                        start=True, stop=True,
                    )
                pt = ptp.tile([128, 1024], FPR, tag="pt", name="pt")
                nc.scalar.activation(pt[:, :], g[:, :], Exp, scale=HD ** -0.5)
                if b >= 4 * a:  # diagonal chunk: zero sk > sq
                    for hh in range(2):
                        nc.gpsimd.affine_select(
                            out=pt[:, hh * 512:(hh + 1) * 512],
                            in_=pt[:, hh * 512:(hh + 1) * 512],
                            compare_op=mybir.AluOpType.is_ge,
                            fill=0.0, base=a * 512 - b * 128,
                            pattern=[[1, 512]], channel_multiplier=-1,
                        )
                for hh in range(2):
                    h = 2 * ht + hh
                    nc.tensor.matmul(
                        oTs[hh][:, :],
                        lhsT=v_sb[b][:, h * 65:(h + 1) * 65],
                        rhs=pt[:, hh * 512:(hh + 1) * 512],
                        start=(b == 0), stop=(b == nchunks - 1),
                    )

            def att_head_tail(a, ht, hh, oTs, onat):
                h = 2 * ht + hh
                oT_sb = osb.tile([HD + 1, 512], FP, tag="oTsb", name="oTsb")
                nc.vector.tensor_copy(oT_sb[:, :], oTs[hh][:, :])
                for c in range(4):
                    tr = gp.tile([128, HD + 1], mybir.dt.float32,
                                 tag="G", name="tr")
                    nc.tensor.transpose(
                        tr[:, :HD + 1], oT_sb[:, c * 128:(c + 1) * 128],
                        ident[:HD + 1, :HD + 1],
                    )
                    recip = rcp.tile([128, 1], FP, tag="recip", name="recip")
                    nc.vector.reciprocal(recip[:, :], tr[:, HD:HD + 1])
                    nc.vector.tensor_scalar_mul(
                        onat[c][:, h * 64:(h + 1) * 64], tr[:, :HD], recip[:, :]
                    )

            # ---- phase 1: the minimum needed by stripe a=0 head 0 ----
            proj_qk_unit(wq_k, bq_sb, qT_sb, 0, 0)
            proj_qk_unit(wk_k, bk_sb, kT_sb, 0, 0)
            for c in range(4):
                proj_v_unit(c)

            # remaining projection units are doled out between attention
            # units, scheduled well before their first consumer, to keep the
            # PE busy while ACT works through the exp stream
            def qk0(wt, bt, dst, t):
                return lambda: proj_qk_unit(wt, bt, dst, t, 0)

            def qk1(wt, bt, dst, t):
                return lambda: proj_qk_unit(wt, bt, dst, t, 1)

            filler = {
                0: qk0(wq_k, bq_sb, qT_sb, 1), 1: qk0(wk_k, bk_sb, kT_sb, 1),
                2: lambda: proj_v_unit(4), 3: lambda: proj_v_unit(5),
                5: lambda: proj_v_unit(6), 7: lambda: proj_v_unit(7),
                9: qk1(wq_k, bq_sb, qT_sb, 0), 11: qk1(wk_k, bk_sb, kT_sb, 0),
                13: qk1(wq_k, bq_sb, qT_sb, 1), 15: qk1(wk_k, bk_sb, kT_sb, 1),
                17: lambda: proj_v_unit(8), 19: lambda: proj_v_unit(9),
                21: lambda: proj_v_unit(10), 23: lambda: proj_v_unit(11),
                30: lambda: proj_v_unit(12), 36: lambda: proj_v_unit(13),
                42: lambda: proj_v_unit(14), 48: lambda: proj_v_unit(15),
            }

            # ---- phases 2+3: attention ----
            uidx = 0
            for a in range(NA):
                onat = [onp.tile([128, WC], FP, tag="onat", name="onat")
                        for _ in range(4)]
                nchunks = 4 * a + 4
                for ht in range(2):
                    oTs = [otp.tile([HD + 1, 512], mybir.dt.float32,
                                    tag="oT", name="oT") for _ in range(2)]
                    for b in range(nchunks):
                        att_unit(a, ht, b, nchunks, oTs)
                        if uidx in filler:
                            filler[uidx]()
                        uidx += 1
                    for hh in range(2):
                        att_head_tail(a, ht, hh, oTs, onat)
                for c in range(4):
                    r = (a * 4 + c) * 128
                    nc.sync.dma_start(out[r:r + 128, :], onat[c][:, :])

    nc.compile()
    return nc


def _get_nc():
    global _NC
    if _NC is None:
        _NC = _build()
    return _NC


def make_in_maps(hidden_states, Wqk, bqk, Wv, bv):
    x = np.ascontiguousarray(np.asarray(hidden_states, dtype=np.float32))
    Wqk = np.asarray(Wqk, dtype=np.float32)
    bqk = np.asarray(bqk, dtype=np.float32)
    Wv = np.asarray(Wv, dtype=np.float32)
    bv = np.asarray(bv, dtype=np.float32)

    xTs = [np.ascontiguousarray(x[b].T) for b in range(B)]
    in_maps = []
    for c in range(8):
        b, ho = c // 4, (c % 4) * NHL
        cols = slice(ho * HD, (ho + NHL) * HD)
        wv_aug = np.zeros((HID + 1, VC), np.float32)
        for h in range(NHL):
            wv_aug[:HID, h * 65:h * 65 + HD] = Wv[:, (ho + h) * HD:(ho + h + 1) * HD]
            wv_aug[HID, h * 65:h * 65 + HD] = bv[(ho + h) * HD:(ho + h + 1) * HD]
            wv_aug[HID, h * 65 + HD] = 1.0
        in_maps.append({
            "xT": xTs[b],
            "wq": np.ascontiguousarray(Wqk[:, cols]),
            "wk": np.ascontiguousarray(Wqk[:, HID:][:, cols]),
            "wv": wv_aug,
            "bq": np.ascontiguousarray(bqk[:HID][cols].reshape(WC, 1)),
            "bk": np.ascontiguousarray(bqk[HID:][cols].reshape(WC, 1)),
            "ones": np.ones((1, 128), np.float32),
        })
    return in_maps


def kernel(hidden_states, Wqk, bqk, Wv, bv):
    from concourse.bass_utils import run_bass_kernel_spmd

    in_maps = make_in_maps(hidden_states, Wqk, bqk, Wv, bv)
    res = run_bass_kernel_spmd(_get_nc(), in_maps, list(range(8)))
    outp = np.empty((B, S, NH * HD), np.float32)
    for c in range(8):
        b, ho = c // 4, (c % 4) * NHL
        outp[b, :, ho * HD:(ho + NHL) * HD] = res.results[c]["out"]
    return outp


# revision 21
# speedup vs baseline: 1.1475x; 1.1475x over previous
"""Causal self-attention (B=2, S=2048, HID=1024, 16 heads x 64) on 8 trn2
NeuronCores.

Sharding: data-parallel over batch (cores 0-3 -> batch 0, cores 4-7 ->
batch 1), tensor-parallel over heads (4 heads per core via Wqk/Wv column
slices). Each core computes its 4 heads end-to-end; the [S, S] score
matrix stays core-local.

Per-core layout choices:
  - q, k are produced TRANSPOSED ([head_cols, S]) so score matmuls need
    no on-device transposes; scores are computed transposed ([sk, sq])
    so the P @ v matmul consumes exp(scores) directly from SBUF.
  - v carries an appended ones-column per head; the attention output
    matmul then yields softmax row-sums in an extra partition row for
    free (no max-subtraction is needed: scores are O(5) so exp is safe
    in fp32, and masked entries are zeroed multiplicatively post-exp).
  - All matmuls run in float32r (fp32 data, PE fast path).
  - Heads are processed in pairs: the two K=64 score matmuls sit in PE
    row-groups 0-63 / 64-127 and run concurrently in the array.
  - The second half (S columns 1024:2048) of the q/k/v projections is
    emitted interleaved with the attention over stripes a=0,1 (which
    only need the first half), so the tensor engine never idles while
    the scalar engine works through the exp() stream -- idle windows
    re-throttle the PE clock to 1.2 GHz (HAM).
"""
import sys

for _p in ("/opt/trn_rl_repo",):
    if _p not in sys.path:
        sys.path.insert(0, _p)

import numpy as np

B, S, HID = 2, 2048, 1024
NH, HD = 16, 64
NHL = 4            # heads per core
WC = NHL * HD      # 256 local q/k weight cols
VC = NHL * (HD + 1)  # 260 local v cols incl. ones col
NT = S // 128      # 16 key chunks
NA = S // 512      # 4 query stripes
NK = HID // 128    # 8 contraction chunks

_NC = None


def _build():
    from concourse import bacc, mybir
    from concourse.tile import TileContext
    from concourse.masks import make_identity

    FP = mybir.dt.float32
    FPR = mybir.dt.float32r
    Exp = mybir.ActivationFunctionType.Exp

    nc = bacc.Bacc("TRN2", target_bir_lowering=False, debug=False, num_devices=8)

    xT = nc.dram_tensor("xT", [HID, S], FPR, kind="ExternalInput")
    wq = nc.dram_tensor("wq", [HID, WC], FPR, kind="ExternalInput")
    wk = nc.dram_tensor("wk", [HID, WC], FPR, kind="ExternalInput")
    wv = nc.dram_tensor("wv", [HID + 1, VC], FPR, kind="ExternalInput")
    bq = nc.dram_tensor("bq", [WC, 1], FP, kind="ExternalInput")
    bk = nc.dram_tensor("bk", [WC, 1], FP, kind="ExternalInput")
    ones = nc.dram_tensor("ones", [1, 128], FPR, kind="ExternalInput")
    out = nc.dram_tensor("out", [S, WC], FP, kind="ExternalOutput")

    with TileContext(nc) as tc:
        with (
            tc.tile_pool(name="inp", bufs=1) as inp,
            tc.tile_pool(name="ptp", bufs=3) as ptp,
            tc.tile_pool(name="osb", bufs=16) as osb,
            tc.tile_pool(name="rcp", bufs=4) as rcp,
            tc.tile_pool(name="onat", bufs=8) as onp,
            tc.tile_pool(name="G", bufs=3, space="PSUM") as gp,
            tc.tile_pool(name="oT", bufs=2, space="PSUM") as otp,
        ):
            # ---- persistent inputs in SBUF ----
            wq_k = [inp.tile([128, WC], FPR, name=f"wq{k}") for k in range(NK)]
            for k in range(NK):
                nc.sync.dma_start(wq_k[k][:, :], wq[k * 128:(k + 1) * 128, :])
            # x.T, column-grouped (nth = 1024-wide halves of S) so the first
            # projection matmuls start after 4MB of DMA, not 8MB
            xk = [[None] * 2 for _ in range(NK)]
            for nth in range(2):
                for k in range(NK):
                    t = inp.tile([128, 1024], FPR, name=f"x{k}_{nth}")
                    nc.sync.dma_start(
                        t[:, :], xT[k * 128:(k + 1) * 128, nth * 1024:(nth + 1) * 1024]
                    )
                    xk[k][nth] = t
                if nth == 0:
                    wk_k = [inp.tile([128, WC], FPR, name=f"wk{k}") for k in range(NK)]
                    for k in range(NK):
                        nc.sync.dma_start(wk_k[k][:, :], wk[k * 128:(k + 1) * 128, :])
                    # v weights must land before the first-half v projections
                    # (they gate attention stripe a=0)
                    wv_k = [inp.tile([128, VC], FPR, name=f"wv{k}")
                            for k in range(NK)]
                    for k in range(NK):
                        nc.sync.dma_start(wv_k[k][:, :], wv[k * 128:(k + 1) * 128, :])
                    wv_last = inp.tile([1, VC], FPR, name="wvl")
                    nc.sync.dma_start(wv_last[:, :], wv[HID:HID + 1, :])
            bq_sb = [inp.tile([128, 1], FP, name=f"bq{t}") for t in range(2)]
            bk_sb = [inp.tile([128, 1], FP, name=f"bk{t}") for t in range(2)]
            for t in range(2):
                nc.sync.dma_start(bq_sb[t][:, :], bq[t * 128:(t + 1) * 128, :])
                nc.sync.dma_start(bk_sb[t][:, :], bk[t * 128:(t + 1) * 128, :])
            ones1 = inp.tile([1, 128], FPR, name="ones1")
            nc.sync.dma_start(ones1[:, :], ones[:, :])
            ident = inp.tile([128, 128], FP, name="ident")
            make_identity(nc, ident[:, :])

            # split by S-half (nth) so interleaved second-half projection
            # writes can't false-depend against first-half attention reads
            qT_sb = [[inp.tile([128, 1024], FPR, name=f"qT{t}_{n}")
                      for n in range(2)] for t in range(2)]
            kT_sb = [[inp.tile([128, 1024], FPR, name=f"kT{t}_{n}")
                      for n in range(2)] for t in range(2)]
            v_sb = [inp.tile([128, VC], FPR, name=f"v{c}") for c in range(NT)]

            # ---- projection emitters ----
            def proj_qk_unit(wt, bt, dst, t, nth):
                g = gp.tile([128, 1024], mybir.dt.float32, tag="G", name="g")
                for k in range(NK):
                    for sub in range(2):
                        nc.tensor.matmul(
                            g[:, sub * 512:(sub + 1) * 512],
                            lhsT=wt[k][:, t * 128:(t + 1) * 128],
                            rhs=xk[k][nth][:, sub * 512:(sub + 1) * 512],
                            start=(k == 0), stop=(k == NK - 1),
                        )
                nc.vector.tensor_scalar_add(
                    dst[t][nth][:, :], g[:, :], bt[t][:, :]
                )

            def proj_v_unit(c):
                nth, cc = divmod(c, 8)
                g = gp.tile([128, 1024], mybir.dt.float32, tag="G", name="g")
                for k in range(NK):
                    nc.tensor.matmul(
                        g[:, :VC],
                        lhsT=xk[k][nth][:, cc * 128:(cc + 1) * 128],
                        rhs=wv_k[k][:, :],
                        start=(k == 0), stop=False,
                    )
                nc.tensor.matmul(  # bias row + ones column (K=1)
                    g[:, :VC], lhsT=ones1[:, :], rhs=wv_last[:, :],
                    start=False, stop=True,
                )
                nc.vector.tensor_copy(v_sb[c][:, :], g[:, :VC])

            # ---- attention emitters ----
            # unit = ONE key chunk b for a head PAIR: g = [h0-slice | h1-slice],
            # one exp covers both heads; fine granularity keeps 3 chunks in
            # flight within the 6 PSUM banks of the G pool
            def att_unit(a, ht, b, nchunks, oTs):
                g = gp.tile([128, 1024], mybir.dt.float32, tag="G", name="g")
                kn, ko = divmod(b * 128, 1024)
                qn, qo = divmod(a * 512, 1024)
                for hh in range(2):
                    hb = hh * 64
                    nc.tensor.matmul(
                        g[:, hh * 512:(hh + 1) * 512],
                        lhsT=kT_sb[ht][kn][hb:hb + 64, ko:ko + 128],
                        rhs=qT_sb[ht][qn][hb:hb + 64, qo:qo + 512],
                        start=True, stop=True,
                    )
                pt = ptp.tile([128, 1024], FPR, tag="pt", name="pt")
                nc.scalar.activation(pt[:, :], g[:, :], Exp, scale=HD ** -0.5)
                if b >= 4 * a:  # diagonal chunk: zero sk > sq
                    for hh in range(2):
                        nc.gpsimd.affine_select(
                            out=pt[:, hh * 512:(hh + 1) * 512],
                            in_=pt[:, hh * 512:(hh + 1) * 512],
                            compare_op=mybir.AluOpType.is_ge,
                            fill=0.0, base=a * 512 - b * 128,
                            pattern=[[1, 512]], channel_multiplier=-1,
                        )
                for hh in range(2):
                    h = 2 * ht + hh
                    nc.tensor.matmul(
                        oTs[hh][:, :],
                        lhsT=v_sb[b][:, h * 65:(h + 1) * 65],
                        rhs=pt[:, hh * 512:(hh + 1) * 512],
                        start=(b == 0), stop=(b == nchunks - 1),
                    )

            def finish_head(a, ht, hh, oT_sb, onat):
                h = 2 * ht + hh
                for c in range(4):
                    tr = gp.tile([128, HD + 1], mybir.dt.float32,
                                 tag="G", name="tr")
                    nc.tensor.transpose(
                        tr[:, :HD + 1], oT_sb[:, c * 128:(c + 1) * 128],
                        ident[:HD + 1, :HD + 1],
                    )
                    recip = rcp.tile([128, 1], FP, tag="recip", name="recip")
                    nc.vector.reciprocal(recip[:, :], tr[:, HD:HD + 1])
                    nc.vector.tensor_scalar_mul(
                        onat[c][:, h * 64:(h + 1) * 64], tr[:, :HD], recip[:, :]
                    )

            # ---- phase 1: the minimum needed by stripe a=0 head pair 0 ----
            proj_qk_unit(wq_k, bq_sb, qT_sb, 0, 0)
            proj_qk_unit(wk_k, bk_sb, kT_sb, 0, 0)
            proj_v_unit(0)
            proj_v_unit(1)

            # remaining projection units are doled out between attention
            # units, scheduled (just) before their first consumer, keeping
            # the PE busy while ACT works through the exp stream
            def qk0(wt, bt, dst, t):
                return lambda: proj_qk_unit(wt, bt, dst, t, 0)

            def qk1(wt, bt, dst, t):
                return lambda: proj_qk_unit(wt, bt, dst, t, 1)

            filler = {
                0: lambda: proj_v_unit(2), 1: lambda: proj_v_unit(3),
                2: qk0(wq_k, bq_sb, qT_sb, 1), 3: qk0(wk_k, bk_sb, kT_sb, 1),
                6: lambda: proj_v_unit(4), 8: lambda: proj_v_unit(5),
                10: lambda: proj_v_unit(6), 11: lambda: proj_v_unit(7),
                14: qk1(wq_k, bq_sb, qT_sb, 0), 17: qk1(wk_k, bk_sb, kT_sb, 0),
                20: qk1(wq_k, bq_sb, qT_sb, 1), 23: qk1(wk_k, bk_sb, kT_sb, 1),
                26: lambda: proj_v_unit(8), 28: lambda: proj_v_unit(9),
                30: lambda: proj_v_unit(10), 31: lambda: proj_v_unit(11),
                38: lambda: proj_v_unit(12), 42: lambda: proj_v_unit(13),
                46: lambda: proj_v_unit(14), 50: lambda: proj_v_unit(15),
            }

            # ---- phases 2+3: attention (tails deferred to the end) ----
            done_heads = []
            uidx = 0
            for a in range(NA):
                nchunks = 4 * a + 4
                for ht in range(2):
                    oTs = [otp.tile([HD + 1, 512], mybir.dt.float32,
                                    tag="oT", name="oT") for _ in range(2)]
                    for b in range(nchunks):
                        att_unit(a, ht, b, nchunks, oTs)
                        if uidx in filler:
                            filler[uidx]()
                        uidx += 1
                    # drain oT psum quickly so the next head pair can start;
                    # transposes/normalization happen after all attention
                    for hh in range(2):
                        oT_sb = osb.tile([HD + 1, 512], FP, tag="oTsb",
                                         name="oTsb")
                        nc.vector.tensor_copy(oT_sb[:, :], oTs[hh][:, :])
                        done_heads.append((a, ht, hh, oT_sb))

            # ---- tail: transpose + normalize + store, one stripe at a time ----
            for a in range(NA):
                onat = [onp.tile([128, WC], FP, tag="onat", name="onat")
                        for _ in range(4)]
                for aa, ht, hh, oT_sb in done_heads:
                    if aa == a:
                        finish_head(a, ht, hh, oT_sb, onat)
                for c in range(4):
                    r = (a * 4 + c) * 128
                    nc.sync.dma_start(out[r:r + 128, :], onat[c][:, :])

    nc.compile()
    return nc


def _get_nc():
    global _NC
    if _NC is None:
        _NC = _build()
    return _NC


def make_in_maps(hidden_states, Wqk, bqk, Wv, bv):
    x = np.ascontiguousarray(np.asarray(hidden_states, dtype=np.float32))
    Wqk = np.asarray(Wqk, dtype=np.float32)
    bqk = np.asarray(bqk, dtype=np.float32)
    Wv = np.asarray(Wv, dtype=np.float32)
    bv = np.asarray(bv, dtype=np.float32)

    xTs = [np.ascontiguousarray(x[b].T) for b in range(B)]
    in_maps = []
    for c in range(8):
        b, ho = c // 4, (c % 4) * NHL
        cols = slice(ho * HD, (ho + NHL) * HD)
        wv_aug = np.zeros((HID + 1, VC), np.float32)
        for h in range(NHL):
            wv_aug[:HID, h * 65:h * 65 + HD] = Wv[:, (ho + h) * HD:(ho + h + 1) * HD]
            wv_aug[HID, h * 65:h * 65 + HD] = bv[(ho + h) * HD:(ho + h + 1) * HD]
            wv_aug[HID, h * 65 + HD] = 1.0
        in_maps.append({
            "xT": xTs[b],
            "wq": np.ascontiguousarray(Wqk[:, cols]),
            "wk": np.ascontiguousarray(Wqk[:, HID:][:, cols]),
            "wv": wv_aug,
            "bq": np.ascontiguousarray(bqk[:HID][cols].reshape(WC, 1)),
            "bk": np.ascontiguousarray(bqk[HID:][cols].reshape(WC, 1)),
            "ones": np.ones((1, 128), np.float32),
        })
    return in_maps


def kernel(hidden_states, Wqk, bqk, Wv, bv):
    from concourse.bass_utils import run_bass_kernel_spmd

    in_maps = make_in_maps(hidden_states, Wqk, bqk, Wv, bv)
    res = run_bass_kernel_spmd(_get_nc(), in_maps, list(range(8)))
    outp = np.empty((B, S, NH * HD), np.float32)
    for c in range(8):
        b, ho = c // 4, (c % 4) * NHL
        outp[b, :, ho * HD:(ho + NHL) * HD] = res.results[c]["out"]
    return outp
